# revision 4
# baseline (speedup 1.0000x reference)
"""Variant of kernel.py without cross-device collectives: two-phase BN.

Phase 1 (sharded, 8 cores): compute pre-BN activations + per-shard
sum/sumsq. Host reduces stats. Phase 2 (sharded): apply BN + relu.
"""

import numpy as np
import jax
import jax.numpy as jnp
from jax.sharding import Mesh, PartitionSpec as P
from jax.experimental.shard_map import shard_map

B, CIN, COUT, H, W = 2, 64, 64, 64, 2048
EPS = 1e-5
NCORES = 8
SLABS = 4
SH = H // SLABS

_phase1 = None
_phase2 = None


def _body1(xs, mask_s, w_spatial, b_spatial, w_channel, b_channel, w_agg):
    b, c, hp, wp = xs.shape
    h, w = hp - 2, wp - 2
    # positions (+ norm) for the full padded slab; pn4 field [1,4,hp,wp]
    posf = xs[:, :3]
    rngf = jnp.sqrt(jnp.sum(posf * posf, axis=1, keepdims=True))
    pn4f = jnp.concatenate([posf, rngf], axis=1)          # [1,4,hp,wp]
    ctr = pn4f[:, :, 1:1 + h, 1:1 + w]                    # [1,4,h,w]

    # per-k attention logits from 4-channel position field only
    ws_l, wc_l = [], []
    for di in range(3):
        for dj in range(3):
            d = pn4f[:, :, di:di + h, dj:dj + w] - ctr    # [1,4,h,w]
            lws = jnp.einsum('bmhw,cm->bchw', d, w_spatial) + b_spatial[None, :, None, None]
            lwc = jnp.einsum('bmhw,cm->bchw', d, w_channel) + b_channel[None, :, None, None]
            ws_l.append(jnp.max(lws, axis=1))             # [1,h,w] (max over c)
            wc_l.append(lwc)
    ws = jnp.stack(ws_l, axis=1)                          # [1,9,h,w]
    wc = jnp.max(jnp.stack(wc_l, axis=2), axis=2)         # [1,C,h,w] (max over k)
    ws = jax.nn.softmax(ws, axis=1)                       # softmax over k
    wc = jax.nn.softmax(wc, axis=1)                       # softmax over c

    # aggregate: out[o] = sum_k W_k @ ((ws_k + wc) * m_k * feat_k)
    w_agg9 = w_agg.reshape(COUT, 9, CIN).astype(jnp.bfloat16)
    out = jnp.zeros((1, COUT, h, w), jnp.float32)
    for k in range(9):
        di, dj = k // 3, k % 3
        fk = xs[:, 3:, di:di + h, dj:dj + w]              # [1,C,h,w]
        mk = mask_s[:, :, di:di + h, dj:dj + w]           # [1,1,h,w]
        wk = (ws[:, k:k + 1] + wc) * mk                   # [1,C,h,w]
        sf = (wk * fk).astype(jnp.bfloat16)
        out = out + jnp.einsum('bchw,oc->bohw', sf, w_agg9[:, k],
                               preferred_element_type=jnp.float32)
    out = out.reshape(1, COUT, h * w)
    s1 = jnp.sum(out, axis=(0, 2))
    s2 = jnp.sum(out * out, axis=(0, 2))
    return out.reshape(1, COUT, h, w), s1[None], s2[None]


def _body2(o, a, bvec):
    # o: [1, COUT, SH, W]; a, b: [COUT] replicated
    return jax.nn.relu(o * a[None, :, None, None] + bvec[None, :, None, None])


def _build():
    devs = jax.devices()[:NCORES]
    mesh = Mesh(np.asarray(devs), ('core',))
    p1 = jax.jit(shard_map(
        _body1, mesh=mesh,
        in_specs=(P('core'), P('core'), P(), P(), P(), P(), P()),
        out_specs=(P('core'), P('core'), P('core')),
        check_rep=False,
    ))
    p2 = jax.jit(shard_map(
        _body2, mesh=mesh,
        in_specs=(P('core'), P(), P()),
        out_specs=P('core'),
        check_rep=False,
    ))
    return p1, p2


def kernel(x, mask, w_spatial, b_spatial, w_channel, b_channel, w_agg,
           gamma, beta):
    global _phase1, _phase2
    x = np.asarray(x, np.float32)
    mask_f = np.asarray(mask).astype(np.float32)

    xp = np.pad(x, ((0, 0), (0, 0), (1, 1), (1, 1)))
    mp = np.pad(mask_f, ((0, 0), (0, 0), (1, 1), (1, 1)))
    xs = np.concatenate([xp[b:b + 1, :, s * SH:s * SH + SH + 2, :]
                         for b in range(B) for s in range(SLABS)], axis=0)
    ms = np.concatenate([mp[b:b + 1, :, s * SH:s * SH + SH + 2, :]
                         for b in range(B) for s in range(SLABS)], axis=0)

    if _phase1 is None:
        _phase1, _phase2 = _build()

    o, s1, s2 = _phase1(jnp.asarray(xs), jnp.asarray(ms),
                        jnp.asarray(w_spatial), jnp.asarray(b_spatial),
                        jnp.asarray(w_channel), jnp.asarray(b_channel),
                        jnp.asarray(w_agg))
    s1 = np.asarray(s1).sum(0)
    s2 = np.asarray(s2).sum(0)
    cnt = float(B * H * W)
    mu = s1 / cnt
    var = s2 / cnt - mu * mu
    g = np.asarray(w_agg.dtype.type(0))  # noqa - placeholder no-op
    gamma = np.asarray(gamma)
    beta = np.asarray(beta)
    a = gamma / np.sqrt(var + EPS)
    bvec = beta - mu * a

    out_sh = _phase2(o, jnp.asarray(a.astype(np.float32)),
                     jnp.asarray(bvec.astype(np.float32)))
    out_sh = np.asarray(out_sh)

    out = np.empty((B, COUT, H, W), np.float32)
    i = 0
    for b in range(B):
        for s in range(SLABS):
            out[b, :, s * SH:(s + 1) * SH, :] = out_sh[i]
            i += 1
    return out


# revision 5
# speedup vs baseline: 4.5982x; 4.5982x over previous
"""Variant of kernel.py without cross-device collectives: two-phase BN.

Phase 1 (sharded, 8 cores): compute pre-BN activations + per-shard
sum/sumsq. Host reduces stats. Phase 2 (sharded): apply BN + relu.
"""

import numpy as np
import jax
import jax.numpy as jnp
from jax.sharding import Mesh, PartitionSpec as P
from jax.experimental.shard_map import shard_map

B, CIN, COUT, H, W = 2, 64, 64, 64, 2048
EPS = 1e-5
NCORES = 8
SLABS = 4
SH = H // SLABS

_phase1 = None
_phase2 = None


def _body1(xs, mask_s, w_spatial, b_spatial, w_channel, b_channel, w_agg, gamma, beta):
    b, c, hp, wp = xs.shape
    h, w = hp - 2, wp - 2
    # positions (+ norm) for the full padded slab; pn4 field [1,4,hp,wp]
    posf = xs[:, :3]
    rngf = jnp.sqrt(jnp.sum(posf * posf, axis=1, keepdims=True))
    pn4f = jnp.concatenate([posf, rngf], axis=1)          # [1,4,hp,wp]
    ctr = pn4f[:, :, 1:1 + h, 1:1 + w]                    # [1,4,h,w]

    # per-k attention logits from 4-channel position field only
    ws_l, wc_l = [], []
    for di in range(3):
        for dj in range(3):
            d = pn4f[:, :, di:di + h, dj:dj + w] - ctr    # [1,4,h,w]
            lws = jnp.einsum('bmhw,cm->bchw', d, w_spatial) + b_spatial[None, :, None, None]
            lwc = jnp.einsum('bmhw,cm->bchw', d, w_channel) + b_channel[None, :, None, None]
            ws_l.append(jnp.max(lws, axis=1))             # [1,h,w] (max over c)
            wc_l.append(lwc)
    ws = jnp.stack(ws_l, axis=1)                          # [1,9,h,w]
    wc = jnp.max(jnp.stack(wc_l, axis=2), axis=2)         # [1,C,h,w] (max over k)
    ws = jax.nn.softmax(ws, axis=1)                       # softmax over k
    wc = jax.nn.softmax(wc, axis=1)                       # softmax over c

    # aggregate: out[o] = sum_k W_k @ ((ws_k + wc) * m_k * feat_k)
    w_agg9 = w_agg.reshape(COUT, 9, CIN).astype(jnp.bfloat16)
    out = jnp.zeros((1, COUT, h, w), jnp.float32)
    for k in range(9):
        di, dj = k // 3, k % 3
        fk = xs[:, 3:, di:di + h, dj:dj + w]              # [1,C,h,w]
        mk = mask_s[:, :, di:di + h, dj:dj + w]           # [1,1,h,w]
        wk = (ws[:, k:k + 1] + wc) * mk                   # [1,C,h,w]
        sf = (wk * fk).astype(jnp.bfloat16)
        out = out + jnp.einsum('bchw,oc->bohw', sf, w_agg9[:, k],
                               preferred_element_type=jnp.float32)
    out = out.reshape(1, COUT, h * w)
    s1 = jax.lax.psum(jnp.sum(out, axis=(0, 2)), 'core')
    s2 = jax.lax.psum(jnp.sum(out * out, axis=(0, 2)), 'core')
    cnt = jnp.float32(NCORES * h * w)
    mu = s1 / cnt
    var = s2 / cnt - mu * mu
    a = gamma * jax.lax.rsqrt(var + EPS)
    bvec = beta - mu * a
    o = jax.nn.relu(out * a[None, :, None] + bvec[None, :, None])
    return o.reshape(1, COUT, h, w)


def _body2(o, a, bvec):
    # o: [1, COUT, SH, W]; a, b: [COUT] replicated
    return jax.nn.relu(o * a[None, :, None, None] + bvec[None, :, None, None])


def _build():
    devs = jax.devices()[:NCORES]
    mesh = Mesh(np.asarray(devs), ('core',))
    p1 = jax.jit(shard_map(
        _body1, mesh=mesh,
        in_specs=(P('core'), P('core'), P(), P(), P(), P(), P(), P(), P()),
        out_specs=P('core'),
        check_rep=False,
    ))
    p2 = jax.jit(shard_map(
        _body2, mesh=mesh,
        in_specs=(P('core'), P(), P()),
        out_specs=P('core'),
        check_rep=False,
    ))
    return p1, p2


def kernel(x, mask, w_spatial, b_spatial, w_channel, b_channel, w_agg,
           gamma, beta):
    global _phase1, _phase2
    x = np.asarray(x, np.float32)
    mask_f = np.asarray(mask).astype(np.float32)

    xp = np.pad(x, ((0, 0), (0, 0), (1, 1), (1, 1)))
    mp = np.pad(mask_f, ((0, 0), (0, 0), (1, 1), (1, 1)))
    xs = np.concatenate([xp[b:b + 1, :, s * SH:s * SH + SH + 2, :]
                         for b in range(B) for s in range(SLABS)], axis=0)
    ms = np.concatenate([mp[b:b + 1, :, s * SH:s * SH + SH + 2, :]
                         for b in range(B) for s in range(SLABS)], axis=0)

    if _phase1 is None:
        _phase1, _phase2 = _build()

    out_sh = _phase1(jnp.asarray(xs), jnp.asarray(ms),
                     jnp.asarray(w_spatial), jnp.asarray(b_spatial),
                     jnp.asarray(w_channel), jnp.asarray(b_channel),
                     jnp.asarray(w_agg),
                     jnp.asarray(np.asarray(gamma, np.float32)),
                     jnp.asarray(np.asarray(beta, np.float32)))
    out_sh = np.asarray(out_sh)

    out = np.empty((B, COUT, H, W), np.float32)
    i = 0
    for b in range(B):
        for s in range(SLABS):
            out[b, :, s * SH:(s + 1) * SH, :] = out_sh[i]
            i += 1
    return out
